# revision 30
# baseline (speedup 1.0000x reference)
"""Causal self-attention TRN2 Bass kernel.

Problem: B=2, T=4096, D_MODEL=512, N_HEADS=8, HEAD_DIM=64 (fp32).

Sharding (tensor+data parallel): 8 cores = 2 batches x 4 head-pairs.
Core c handles batch b = c//4 and heads (2g, 2g+1) with g = c%4, over the
full sequence. Each core computes a full-shape [T, 512] partial output
(its two heads' contribution through W_O); the host sums 4 partials per
batch ("unshard" of the tensor-parallel contraction).

Per-core algorithm (flash-style, no max subtraction -- scores stay small
enough that exp() cannot overflow bf16; softmax is exact without the max
trick):

  Everything off the per-chunk critical path is FILLER-PIECE SCHEDULED:
  the QKV projection for token-chunk J+1 (6 pieces: q, k, v x 4) and the
  ENTIRE finalize chain of block J-1 (PSUM drain, sums broadcast,
  reciprocal, normalize, W_O projection, output DMA -- 8 pieces) are
  emitted a few pieces per key-chunk iteration of attention block J.
  Every cross-engine dependency gets a whole block of slack, ScalarE is
  fed from ~10us on (instead of idling behind a 50us serial QKV phase),
  and the PE never idles >3.4us (which would re-throttle the HAM clock
  gate to 1.2GHz -- the failure mode of a serial block boundary).

  qT/kT packed [128, T] (partitions 0:64 head0, 64:128 head1), V_aug
  natural per head [T(part-chunks), 65] with a ones column (the PV matmul
  then accumulates softmax denominators for free).

  Attention per 512-wide query block, over 128-wide key chunks emitted in
  pairs: S^T [k,q] via row-tiled matmul pairs (head0 on PE rows 0:63,
  head1 on rows 64:127, concurrent; consecutive chunks back-to-back so
  LDWEIGHTS overlaps the other row-group's matmul), exp on ScalarE
  (PSUM->SBUF, scale=1/sqrt(64) fused) for ~3/4 of the chunks and on the
  DVE for off-diagonal chunks with K%3==2 via the bit-trick
  bf16_bits(exp(s)) ~= int16(s*A+B) -- one tensor_scalar, fp32->int16
  converts round-to-nearest (HW-verified; +-3% sawtooth on 1/4 of the
  weights, denominators use the same values so softmax normalization is
  consistent). Multiplicative causal mask on diagonal blocks on GpSimd,
  then M=65 PV matmuls per head accumulating [out^T; colsums] in PSUM,
  one chunk-pair behind the scores.

  Normalize late (finalize pieces, running inside the NEXT block): K=1
  outer-product matmuls broadcast the sums rows to 64 partitions, DVE
  reciprocals, per-head multiplies on GpSimd, W_O projection as K=128
  matmuls per 128 queries, fp32 partials DMA'd out.

PSUM budget (8 banks): score tiles [128,1024]x2 = 4, PV accumulators
[65,512]x2 = 2, shared QKV-workspace/broadcast/projection rotation
[128,512]x2 = 2.

NOTE fp8 was tried and REJECTED: e4m3 V + e5m2 P with DoubleRow PV is
~2x faster on the PE but measures 2.8e-2 rel err (gate 2e-2): diffuse
attention shrinks |out| ~ |v|/sqrt(N_eff) while quantization noise stays
at |v| scale, so fp8 noise does not average away relative to the output.
Host-sim confirmed each of (P e5m2), (V e4m3) alone exceeds the gate.
"""

import math
from collections import deque

import ml_dtypes
import numpy as np

import concourse.bass as bass
import concourse.mybir as mybir
import concourse.tile as tile
from concourse.tile import add_dep_helper
from concourse import bacc
from concourse.bass import ds, ts
from concourse.bass_utils import run_bass_kernel_spmd

FP32 = mybir.dt.float32
FP32R = mybir.dt.float32r
BF16 = mybir.dt.bfloat16
I16 = mybir.dt.int16
AF = mybir.ActivationFunctionType

T = 4096
DM = 512
QC = 512  # query-chunk width (free dim)
KC = 128  # key-chunk width (partition dim)

# test.py can flip these before calling kernel()
TRACE = False
LAST_RESULTS = None


def build_program(t=T):
    assert t % QC == 0
    nq = t // QC
    nkc = t // KC
    nc = bacc.Bacc("TRN2", target_bir_lowering=False, debug=False)

    xT = nc.dram_tensor("xT", [DM, t], BF16, kind="ExternalInput").ap()
    # host pre-arranges weights so every DMA is contiguous per partition
    wq = nc.dram_tensor("wq", [128, DM], BF16, kind="ExternalInput").ap()
    wk = nc.dram_tensor("wk", [128, DM], BF16, kind="ExternalInput").ap()
    wv = nc.dram_tensor("wv", [128, DM], BF16, kind="ExternalInput").ap()
    woT = nc.dram_tensor("woT", [128, DM], BF16, kind="ExternalInput").ap()
    outp = nc.dram_tensor("outp", [t, DM], FP32, kind="ExternalOutput").ap()

    with tile.TileContext(nc) as tc:
        with (
            tc.tile_pool(name="consts", bufs=1) as cpool,
            tc.tile_pool(name="persist", bufs=1) as ppool,
            tc.tile_pool(name="xtl", bufs=2) as xpool,
            tc.tile_pool(name="work", bufs=3) as wpool,
            tc.tile_pool(name="ps_sc", bufs=2, space="PSUM") as ps_sc,
            tc.tile_pool(name="ps_pv", bufs=1, space="PSUM") as ps_pv,
            # shared rotation for QKV projection workspace AND the per-Q
            # broadcast/output-projection tiles (all are filler pieces);
            # NOTE a merged 3-deep score+workspace rotation was tried and
            # REGRESSED (211us vs 201us): it couples the score pipeline's
            # WAR to filler-piece readers (DVE copies), which stalls more
            # than the 2-buf exp WAR it was meant to relax.
            tc.tile_pool(name="ps_mi", bufs=2, space="PSUM") as ps_mi,
        ):
            # ---- constants ----
            wq_s = cpool.tile([128, 512], BF16, name="wq_s")
            wk_s = cpool.tile([128, 512], BF16, name="wk_s")
            wv_s = cpool.tile([128, 512], BF16, name="wv_s")
            woT_s = cpool.tile([128, 512], BF16, name="woT_s")
            # startup critical path: the first scores need wq, wk and the
            # first x-chunks. Split across both HWDGE rings (wq/wv/woT on
            # Act, wk ahead of the x-chunks on SP); wv/woT aren't needed
            # until the first v-piece/finalize so they go out last.
            nc.scalar.dma_start(wq_s[:], wq[:])
            nc.sync.dma_start(wk_s[:], wk[:])
            nc.scalar.dma_start(wv_s[:], wv[:])
            nc.scalar.dma_start(woT_s[:], woT[:])

            # multiplicative causal mask for diagonal blocks of P^T [k, q]:
            # 1 where k <= q, 0 elsewhere (applied to exp output on GpSimd)
            mask_s = cpool.tile([128, 128], BF16, name="mask_s")
            nc.gpsimd.memset(mask_s[:], 0.0)
            nc.gpsimd.affine_select(
                out=mask_s[:],
                in_=mask_s[:],
                compare_op=mybir.AluOpType.is_gt,
                fill=1.0,
                base=0,
                # keep 0.0 where (k - q) > 0, fill 1.0 where k <= q
                pattern=[[-1, 128]],
                channel_multiplier=1,
            )

            # ones row at partition 64 for the K=1 reciprocal broadcast
            # (partition 64 so it aligns with the PV sums row)
            ones_row = cpool.tile([65, 64], FP32R, name="ones_row")
            nc.vector.memset(ones_row[:].bitcast(FP32), 1.0)

            # prefetch the exp ACT table set during initial DMA wait
            # (~2.7us table load otherwise lands on the first real exp)
            dum = cpool.tile([1, 8], FP32, name="dum")
            nc.scalar.activation(
                dum[0:1, 0:1], ones_row[:].bitcast(FP32)[0:1, 0:1], AF.Exp
            )

            # ---- persistent activations ----
            # qT/kT packed: partitions 0:64 = head0 dims, 64:128 = head1
            qT_s = ppool.tile([128, t], BF16, name="qT_s")
            kT_s = ppool.tile([128, t], BF16, name="kT_s")
            # V_aug natural: partition = token within key-chunk; per chunk
            # 65 columns = 64 dims + ones (memset once to 1.0; projection
            # copies overwrite the first 64 columns of each chunk).
            # NOTE: fp8 (e4m3 V / e5m2 P) with DoubleRow PV was tried and is
            # 2x faster on the PE, but fails the 2e-2 gate: diffuse attention
            # shrinks |out| ~ |v|/sqrt(N_eff) while quantization noise stays
            # at |v| scale -> measured 2.8e-2 rel err. PV stays bf16.
            v0_s = ppool.tile([128, nkc * 65], BF16, name="v0_s")
            v1_s = ppool.tile([128, nkc * 65], BF16, name="v1_s")
            # unnormalized attention output (transposed) + sums row 64,
            # copied out of PSUM per q-chunk so the PV banks free quickly
            aoU0_s = ppool.tile([65, t], FP32R, name="aoU0_s")
            aoU1_s = ppool.tile([65, t], FP32R, name="aoU1_s")
            nc.vector.memset(v0_s[:], 1.0)
            nc.vector.memset(v1_s[:], 1.0)

            # ---- QKV projection pieces (interleaved into attention) ----
            def make_pieces(tcx):
                xts = []
                for d in range(4):
                    xt = xpool.tile([128, 512], BF16, tag=f"xt{d}", name=f"xt{d}")
                    nc.sync.dma_start(xt[:], xT[ts(d, 128), ts(tcx, 512)])
                    xts.append(xt)
                state = {}

                def qk_piece(wsrc, dst):
                    def go():
                        psp = ps_mi.tile(
                            [128, 512], FP32, tag="mi", name="psp"
                        )
                        for d in range(4):
                            nc.tensor.matmul(
                                psp[:],
                                lhsT=wsrc[:, ts(d, 128)],
                                rhs=xts[d][:],
                                start=(d == 0),
                                stop=(d == 3),
                            )
                        nc.vector.tensor_copy(dst[:, ts(tcx, 512)], psp[:])

                    return go

                def v_piece(tt):
                    def go():
                        if tt == 0:
                            state["psv"] = ps_mi.tile(
                                [128, 512], FP32, tag="mi", name="psv"
                            )
                        psv = state["psv"]
                        for d in range(4):
                            nc.tensor.matmul(
                                psv[:, ts(tt, 128)],
                                lhsT=xts[d][:, ts(tt, 128)],
                                rhs=wv_s[:, ts(d, 128)],
                                start=(d == 0),
                                stop=(d == 3),
                                skip_group_check=True,
                            )
                        if tt == 3:
                            # psv holds chunks kk=4*tcx+tt as [k 4, x 128]
                            src = psv[:].rearrange("p (k x) -> p k x", k=4)
                            nc.vector.tensor_copy(
                                v0_s[:]
                                .rearrange("p (k c) -> p k c", c=65)[
                                    :, ds(tcx * 4, 4), 0:64
                                ],
                                src[:, :, 0:64],
                            )
                            nc.vector.tensor_copy(
                                v1_s[:]
                                .rearrange("p (k c) -> p k c", c=65)[
                                    :, ds(tcx * 4, 4), 0:64
                                ],
                                src[:, :, 64:128],
                            )

                    return go

                return [
                    qk_piece(wq_s, qT_s),
                    qk_piece(wk_s, kT_s),
                    v_piece(0),
                    v_piece(1),
                    v_piece(2),
                    v_piece(3),
                ]

            # ---- per-Q finalize pieces (normalize + W_O projection) ----
            # Deferred into the NEXT q-block's chunk loop as filler pieces so
            # the multi-engine chain (DVE copy -> PE bcast -> DVE recip ->
            # GpSimd mul -> DMA shift -> PE proj -> DVE copy -> DMA out)
            # overlaps a whole block instead of stalling PE/ScalarE at the
            # boundary (a >3.4us PE idle there re-throttles HAM to 1.2GHz).
            def make_finalize(Q, po0, po1):
                qsl = ts(Q, 512)
                state = {}

                def p_copy():
                    # free the PV banks: single DVE copy per head to SBUF,
                    # then broadcast the sums rows to 64 partitions (K=1
                    # matmuls) and take reciprocals -- 64 lanes instead of 1
                    nc.vector.tensor_copy(aoU0_s[:, qsl], po0[:])
                    nc.vector.tensor_copy(aoU1_s[:, qsl], po1[:])

                def p_bcast0():
                    psb0 = ps_mi.tile([64, 512], FP32, tag="mi", name="psb0")
                    nc.tensor.matmul(
                        psb0[:],
                        lhsT=ones_row[64:65, :],
                        rhs=aoU0_s[64:65, qsl],
                        start=True,
                        stop=True,
                    )
                    rbc0 = wpool.tile([64, 512], FP32, tag="bc", name="rbc0")
                    nc.vector.reciprocal_approx_fast(rbc0[:], psb0[:])
                    state["rbc0"] = rbc0

                def p_bcast1():
                    psb1 = ps_mi.tile([64, 512], FP32, tag="mi", name="psb1")
                    nc.tensor.matmul(
                        psb1[:],
                        lhsT=ones_row[64:65, :],
                        rhs=aoU1_s[64:65, qsl],
                        start=True,
                        stop=True,
                    )
                    rbc1 = wpool.tile([64, 512], FP32, tag="bc", name="rbc1")
                    nc.vector.reciprocal_approx_fast(rbc1[:], psb1[:])
                    state["rbc1"] = rbc1

                def p_norm():
                    # normalized attention-out, both heads in one [128, 512]
                    # tile (head1 lands via an SBUF->SBUF DMA partition
                    # shift) so the output projection is a single K=128
                    # matmul per 128 queries; multiplies on GpSimd (idle-ish)
                    aoT_b = wpool.tile([128, 512], BF16, tag="ao", name="aoT_b")
                    nc.gpsimd.tensor_mul(
                        aoT_b[0:64, :],
                        aoU0_s[0:64, qsl].bitcast(FP32),
                        state["rbc0"][:],
                    )
                    aoT1 = wpool.tile([64, 512], BF16, tag="ao1", name="aoT1")
                    nc.gpsimd.tensor_mul(
                        aoT1[:], aoU1_s[0:64, qsl].bitcast(FP32), state["rbc1"][:]
                    )
                    nc.sync.dma_start(aoT_b[64:128, :], aoT1[:])
                    state["aoT_b"] = aoT_b

                def p_proj(qq):
                    def go():
                        pso = ps_mi.tile([128, 512], FP32, tag="mi", name="pso")
                        nc.tensor.matmul(
                            pso[:],
                            lhsT=state["aoT_b"][:, ts(qq, 128)],
                            rhs=woT_s[:],
                            start=True,
                            stop=True,
                        )
                        osb = wpool.tile([128, 512], FP32, tag="os", name="osb")
                        nc.vector.tensor_copy(osb[:], pso[:])
                        nc.sync.dma_start(
                            outp[ds(Q * 512 + qq * 128, 128), :], osb[:]
                        )

                    return go

                return [p_copy, p_bcast0, p_bcast1, p_norm] + [
                    p_proj(qq) for qq in range(4)
                ]

            fillers = deque()
            pieces0 = make_pieces(0)
            for p in pieces0[:2]:
                p()  # q and k of chunk 0: the critical path to first scores
            # v pieces of chunk 0 are only needed by the first PV (two
            # iterations in) -- queue them as fillers ahead of chunk 1's
            fillers.extend(pieces0[2:])
            if nq > 1:
                fillers.extend(make_pieces(1))

            def pop_filler(slots_left):
                # adaptive: drain the queue evenly over the remaining slots
                n = -(-len(fillers) // max(slots_left, 1))
                for _ in range(min(n, len(fillers))):
                    fillers.popleft()()

            # ---- attention ----
            inv_sqrt_d = 1.0 / math.sqrt(64.0)
            # bit-trick exp for the DVE offload path: bf16 bit pattern of
            # exp(s*inv_sqrt_d) ~= int16(s * A7 + B7) (fp32->int16 converts
            # round-to-nearest, HW-verified; +-3% sawtooth error on ~1/4 of
            # the attention weights, and the softmax denominator uses the
            # same approximated values so normalization is consistent;
            # measured end-to-end: 3.2e-3 rel err vs 2.7e-3 all-ScalarE)
            exp_a7 = inv_sqrt_d * 128.0 / math.log(2.0)
            exp_b7 = 16256.0 - 0.0579 * 128.0
            for Q in range(nq):
                po0 = ps_pv.tile([65, 512], FP32, tag="pv0", name="po0")
                po1 = ps_pv.tile([65, 512], FP32, tag="pv1", name="po1")
                nkq = 4 * Q + 4
                pts = {}
                last_scores = None
                # software-pipelined in chunk PAIRS: both chunks' score
                # matmuls are emitted back-to-back (the row-tiled halves
                # alternate PE row groups, so each LDWEIGHTS hides behind
                # the other head's in-flight matmul), then the previous
                # pair's PV matmuls, one pair behind, so the PE never waits
                # out the ScalarE/DVE exp latency
                niter = nkq // 2 + 1
                for I in range(niter):
                    for cpar in (0, 1):
                        K = 2 * I + cpar
                        if K >= nkq:
                            continue
                        off = K * 128 - Q * 512
                        n0 = max(off, 0)
                        w = 512 - n0
                        pssc = ps_sc.tile([128, 1024], FP32, tag="sc", name="pssc")
                        nc.tensor.matmul(
                            pssc[:, n0:512],
                            lhsT=kT_s[0:64, ts(K, 128)],
                            rhs=qT_s[0:64, ds(Q * 512 + n0, w)],
                            start=True,
                            stop=True,
                        )
                        last_scores = nc.tensor.matmul(
                            pssc[:, 512 + n0 : 1024],
                            lhsT=kT_s[64:128, ts(K, 128)],
                            rhs=qT_s[64:128, ds(Q * 512 + n0, w)],
                            start=True,
                            stop=True,
                        )
                        pt = wpool.tile([128, 1024], BF16, tag="pt", name="pt", bufs=4)
                        src = pssc[:].rearrange("p (h n) -> p h n", h=2)[:, :, n0:512]
                        if off < 0 and K % 3 == 2:
                            # offload ~1/3 of the exps to the DVE (ScalarE is
                            # the critical engine); off-diagonal chunks only
                            # so the mask path stays on the ScalarE side
                            dsti = pt[:].bitcast(I16).rearrange(
                                "p (h n) -> p h n", h=2
                            )[:, :, n0:512]
                            nc.vector.tensor_scalar(
                                dsti,
                                src,
                                exp_a7,
                                exp_b7,
                                mybir.AluOpType.mult,
                                mybir.AluOpType.add,
                            )
                        else:
                            dst = pt[:].rearrange("p (h n) -> p h n", h=2)[
                                :, :, n0:512
                            ]
                            nc.scalar.activation(dst, src, AF.Exp, scale=inv_sqrt_d)
                        if off >= 0:
                            # zero the not-yet-valid triangle on the (idle)
                            # GpSimd engine, off the scores->exp chain
                            nc.gpsimd.tensor_mul(
                                pt[:, ds(n0, 128)], pt[:, ds(n0, 128)], mask_s[:]
                            )
                            nc.gpsimd.tensor_mul(
                                pt[:, ds(512 + n0, 128)],
                                pt[:, ds(512 + n0, 128)],
                                mask_s[:],
                            )
                        pts[K] = (pt, n0, w)
                    # filler pieces (QKV projection for block Q+1, finalize
                    # work for block Q-1) spread over the block so every
                    # cross-engine dependency has a whole block of slack and
                    # the PE queue never blocks on a remote chain
                    pop_filler(2 * (niter - I))
                    for cpar in (0, 1):
                        Kp = 2 * (I - 1) + cpar
                        if not (0 <= Kp < nkq):
                            continue
                        pt_p, n0_p, w_p = pts.pop(Kp)
                        st = Kp == 0
                        sp = Kp == nkq - 1
                        pv0_mm = nc.tensor.matmul(
                            po0[0:65, ds(n0_p, w_p)],
                            lhsT=v0_s[:, ds(Kp * 65, 65)],
                            rhs=pt_p[:, ds(n0_p, w_p)],
                            start=st,
                            stop=sp,
                            skip_group_check=True,
                        )
                        if cpar == 0 and I < niter - 1 and last_scores is not None:
                            # order-only edge: keep the PV pairs AFTER the
                            # next chunks' scores on the PE queue so the exp
                            # latency is hidden behind PE work
                            add_dep_helper(
                                pv0_mm.ins,
                                last_scores.ins,
                                sync=False,
                                reason="pipeline skew",
                            )
                        nc.tensor.matmul(
                            po1[0:65, ds(n0_p, w_p)],
                            lhsT=v1_s[:, ds(Kp * 65, 65)],
                            rhs=pt_p[:, ds(512 + n0_p, w_p)],
                            start=st,
                            stop=sp,
                            skip_group_check=True,
                        )
                    pop_filler(2 * (niter - I) - 1)
                # queue this block's finalize + block Q+2's projection as
                # fillers for block Q+1; the last block finalizes inline
                if Q + 1 < nq:
                    fillers.extend(make_finalize(Q, po0, po1))
                    if Q + 2 < nq:
                        fillers.extend(make_pieces(Q + 2))
                else:
                    fillers.extend(make_finalize(Q, po0, po1))
                    while fillers:
                        fillers.popleft()()
    nc.compile()
    return nc


def make_in_maps(x, W_QKV, W_O, t=T, n_cores=8):
    x = np.ascontiguousarray(np.asarray(x, dtype=np.float32))
    W_QKV = np.asarray(W_QKV, dtype=np.float32)
    W_O = np.asarray(W_O, dtype=np.float32)
    B = x.shape[0]
    bf16 = ml_dtypes.bfloat16

    def prearrange(w):
        # w: [128 hd, 512 dm] slice of a linear weight; SBUF wants
        # [p, d*128 + c] = w.T[d*128 + p, c] so the DMA is contiguous
        return np.ascontiguousarray(
            w.T.reshape(4, 128, 128).transpose(1, 0, 2).reshape(128, 512)
        ).astype(bf16)

    xTs = [np.ascontiguousarray(x[b, :t].T).astype(bf16) for b in range(B)]
    in_maps = []
    for c in range(n_cores):
        b = c // 4
        g = c % 4
        hs = slice(2 * g * 64, 2 * g * 64 + 128)
        in_maps.append(
            {
                "xT": xTs[b],
                "wq": prearrange(W_QKV[0:512][hs]),
                "wk": prearrange(W_QKV[512:1024][hs]),
                "wv": prearrange(W_QKV[1024:1536][hs]),
                "woT": np.ascontiguousarray(W_O[:, hs].T).astype(bf16),
            }
        )
    return in_maps


def kernel(x, W_QKV, W_O):
    global LAST_RESULTS
    x = np.asarray(x, dtype=np.float32)
    B, t, _ = x.shape
    nc = build_program(t)
    in_maps = make_in_maps(x, W_QKV, W_O, t=t)
    res = run_bass_kernel_spmd(
        nc, in_maps, core_ids=list(range(8)), trace=TRACE
    )
    LAST_RESULTS = res
    parts = [r["outp"] for r in res.results]
    out = np.empty((B, t, DM), dtype=np.float32)
    for b in range(B):
        acc = np.zeros((t, DM), dtype=np.float64)
        for g in range(4):
            acc += parts[b * 4 + g]
        out[b] = acc.astype(np.float32)
    return out


# revision 33
# speedup vs baseline: 1.0088x; 1.0088x over previous
"""Causal self-attention TRN2 Bass kernel.

Problem: B=2, T=4096, D_MODEL=512, N_HEADS=8, HEAD_DIM=64 (fp32).

Sharding (tensor+data parallel): 8 cores = 2 batches x 4 head-pairs.
Core c handles batch b = c//4 and heads (2g, 2g+1) with g = c%4, over the
full sequence. Each core computes a full-shape [T, 512] partial output
(its two heads' contribution through W_O); the host sums 4 partials per
batch ("unshard" of the tensor-parallel contraction).

Per-core algorithm (flash-style, no max subtraction -- scores stay small
enough that exp() cannot overflow bf16; softmax is exact without the max
trick):

  Everything off the per-chunk critical path is FILLER-PIECE SCHEDULED:
  the QKV projection for token-chunk J+1 (6 pieces: q, k, v x 4) and the
  ENTIRE finalize chain of block J-1 (PSUM drain, sums broadcast,
  reciprocal, normalize, W_O projection, output DMA -- 8 pieces) are
  emitted a few pieces per key-chunk iteration of attention block J.
  Every cross-engine dependency gets a whole block of slack, ScalarE is
  fed from ~10us on (instead of idling behind a 50us serial QKV phase),
  and the PE never idles >3.4us (which would re-throttle the HAM clock
  gate to 1.2GHz -- the failure mode of a serial block boundary).

  qT/kT packed [128, T] (partitions 0:64 head0, 64:128 head1), V_aug
  natural per head [T(part-chunks), 65] with a ones column (the PV matmul
  then accumulates softmax denominators for free).

  Attention per 512-wide query block, over 128-wide key chunks emitted in
  pairs: S^T [k,q] via row-tiled matmul pairs (head0 on PE rows 0:63,
  head1 on rows 64:127, concurrent; consecutive chunks back-to-back so
  LDWEIGHTS overlaps the other row-group's matmul), exp on ScalarE
  (PSUM->SBUF, scale=1/sqrt(64) fused) for ~3/4 of the chunks and on the
  DVE for off-diagonal chunks with K%3==2 via the bit-trick
  bf16_bits(exp(s)) ~= int16(s*A+B) -- one tensor_scalar, fp32->int16
  converts round-to-nearest (HW-verified; +-3% sawtooth on 1/4 of the
  weights, denominators use the same values so softmax normalization is
  consistent). Multiplicative causal mask on diagonal blocks on GpSimd,
  then M=65 PV matmuls per head accumulating [out^T; colsums] in PSUM,
  one chunk-pair behind the scores.

  Normalize late (finalize pieces, running inside the NEXT block): K=1
  outer-product matmuls broadcast the sums rows to 64 partitions, DVE
  reciprocals, per-head multiplies on GpSimd, W_O projection as K=128
  matmuls per 128 queries, fp32 partials DMA'd out.

PSUM budget (8 banks): score tiles [128,1024]x2 = 4, PV accumulators
[65,512]x2 = 2, shared QKV-workspace/broadcast/projection rotation
[128,512]x2 = 2.

NOTE fp8 was tried and REJECTED: e4m3 V + e5m2 P with DoubleRow PV is
~2x faster on the PE but measures 2.8e-2 rel err (gate 2e-2): diffuse
attention shrinks |out| ~ |v|/sqrt(N_eff) while quantization noise stays
at |v| scale, so fp8 noise does not average away relative to the output.
Host-sim confirmed each of (P e5m2), (V e4m3) alone exceeds the gate.
"""

import math
from collections import deque

import ml_dtypes
import numpy as np

import concourse.bass as bass
import concourse.mybir as mybir
import concourse.tile as tile
from concourse.tile import add_dep_helper
from concourse import bacc
from concourse.bass import ds, ts
from concourse.bass_utils import run_bass_kernel_spmd

FP32 = mybir.dt.float32
FP32R = mybir.dt.float32r
BF16 = mybir.dt.bfloat16
I16 = mybir.dt.int16
AF = mybir.ActivationFunctionType

T = 4096
DM = 512
QC = 512  # query-chunk width (free dim)
KC = 128  # key-chunk width (partition dim)

# test.py can flip these before calling kernel()
TRACE = False
LAST_RESULTS = None


def build_program(t=T):
    assert t % QC == 0
    nq = t // QC
    nkc = t // KC
    nc = bacc.Bacc("TRN2", target_bir_lowering=False, debug=False)

    xT = nc.dram_tensor("xT", [DM, t], BF16, kind="ExternalInput").ap()
    # host pre-arranges weights so every DMA is contiguous per partition
    wq = nc.dram_tensor("wq", [128, DM], BF16, kind="ExternalInput").ap()
    wk = nc.dram_tensor("wk", [128, DM], BF16, kind="ExternalInput").ap()
    wv = nc.dram_tensor("wv", [128, DM], BF16, kind="ExternalInput").ap()
    woT = nc.dram_tensor("woT", [128, DM], BF16, kind="ExternalInput").ap()
    outp = nc.dram_tensor("outp", [t, DM], FP32, kind="ExternalOutput").ap()

    with tile.TileContext(nc) as tc:
        with (
            tc.tile_pool(name="consts", bufs=1) as cpool,
            tc.tile_pool(name="persist", bufs=1) as ppool,
            tc.tile_pool(name="xtl", bufs=2) as xpool,
            tc.tile_pool(name="work", bufs=3) as wpool,
            tc.tile_pool(name="ps_sc", bufs=2, space="PSUM") as ps_sc,
            tc.tile_pool(name="ps_pv", bufs=1, space="PSUM") as ps_pv,
            # shared rotation for QKV projection workspace AND the per-Q
            # broadcast/output-projection tiles (all are filler pieces);
            # NOTE a merged 3-deep score+workspace rotation was tried and
            # REGRESSED (211us vs 201us): it couples the score pipeline's
            # WAR to filler-piece readers (DVE copies), which stalls more
            # than the 2-buf exp WAR it was meant to relax.
            tc.tile_pool(name="ps_mi", bufs=2, space="PSUM") as ps_mi,
        ):
            # ---- constants ----
            wq_s = cpool.tile([128, 512], BF16, name="wq_s")
            wk_s = cpool.tile([128, 512], BF16, name="wk_s")
            wv_s = cpool.tile([128, 512], BF16, name="wv_s")
            woT_s = cpool.tile([128, 512], BF16, name="woT_s")
            # weights go out on the second HWDGE ring (Act) so they overlap
            # the first x-chunk loads on the SP ring at startup
            nc.scalar.dma_start(wq_s[:], wq[:])
            nc.scalar.dma_start(wk_s[:], wk[:])
            nc.scalar.dma_start(wv_s[:], wv[:])
            nc.scalar.dma_start(woT_s[:], woT[:])

            # multiplicative causal mask for diagonal blocks of P^T [k, q]:
            # 1 where k <= q, 0 elsewhere (applied to exp output on GpSimd)
            mask_s = cpool.tile([128, 128], BF16, name="mask_s")
            nc.gpsimd.memset(mask_s[:], 0.0)
            nc.gpsimd.affine_select(
                out=mask_s[:],
                in_=mask_s[:],
                compare_op=mybir.AluOpType.is_gt,
                fill=1.0,
                base=0,
                # keep 0.0 where (k - q) > 0, fill 1.0 where k <= q
                pattern=[[-1, 128]],
                channel_multiplier=1,
            )

            # ones row at partition 64 for the K=1 reciprocal broadcast
            # (partition 64 so it aligns with the PV sums row)
            ones_row = cpool.tile([65, 64], FP32R, name="ones_row")
            nc.vector.memset(ones_row[:].bitcast(FP32), 1.0)

            # prefetch the exp ACT table set during initial DMA wait
            # (~2.7us table load otherwise lands on the first real exp)
            dum = cpool.tile([1, 8], FP32, name="dum")
            nc.scalar.activation(
                dum[0:1, 0:1], ones_row[:].bitcast(FP32)[0:1, 0:1], AF.Exp
            )

            # ---- persistent activations ----
            # qT/kT packed: partitions 0:64 = head0 dims, 64:128 = head1
            qT_s = ppool.tile([128, t], BF16, name="qT_s")
            kT_s = ppool.tile([128, t], BF16, name="kT_s")
            # V_aug natural: partition = token within key-chunk; per chunk
            # 65 columns = 64 dims + ones (memset once to 1.0; projection
            # copies overwrite the first 64 columns of each chunk).
            # NOTE: fp8 (e4m3 V / e5m2 P) with DoubleRow PV was tried and is
            # 2x faster on the PE, but fails the 2e-2 gate: diffuse attention
            # shrinks |out| ~ |v|/sqrt(N_eff) while quantization noise stays
            # at |v| scale -> measured 2.8e-2 rel err. PV stays bf16.
            v0_s = ppool.tile([128, nkc * 65], BF16, name="v0_s")
            v1_s = ppool.tile([128, nkc * 65], BF16, name="v1_s")
            # unnormalized attention output (transposed) + sums row 64,
            # copied out of PSUM per q-chunk so the PV banks free quickly
            aoU0_s = ppool.tile([65, t], FP32R, name="aoU0_s")
            aoU1_s = ppool.tile([65, t], FP32R, name="aoU1_s")
            nc.vector.memset(v0_s[:], 1.0)
            nc.vector.memset(v1_s[:], 1.0)

            # ---- QKV projection pieces (interleaved into attention) ----
            def make_pieces(tcx):
                xts = []
                for d in range(4):
                    xt = xpool.tile([128, 512], BF16, tag=f"xt{d}", name=f"xt{d}")
                    nc.sync.dma_start(xt[:], xT[ts(d, 128), ts(tcx, 512)])
                    xts.append(xt)
                state = {}

                def qk_piece(wsrc, dst):
                    def go():
                        psp = ps_mi.tile(
                            [128, 512], FP32, tag="mi", name="psp"
                        )
                        for d in range(4):
                            nc.tensor.matmul(
                                psp[:],
                                lhsT=wsrc[:, ts(d, 128)],
                                rhs=xts[d][:],
                                start=(d == 0),
                                stop=(d == 3),
                            )
                        nc.vector.tensor_copy(dst[:, ts(tcx, 512)], psp[:])

                    return go

                def v_piece(tt):
                    def go():
                        if tt == 0:
                            state["psv"] = ps_mi.tile(
                                [128, 512], FP32, tag="mi", name="psv"
                            )
                        psv = state["psv"]
                        for d in range(4):
                            nc.tensor.matmul(
                                psv[:, ts(tt, 128)],
                                lhsT=xts[d][:, ts(tt, 128)],
                                rhs=wv_s[:, ts(d, 128)],
                                start=(d == 0),
                                stop=(d == 3),
                                skip_group_check=True,
                            )
                        if tt == 3:
                            # psv holds chunks kk=4*tcx+tt as [k 4, x 128]
                            src = psv[:].rearrange("p (k x) -> p k x", k=4)
                            nc.vector.tensor_copy(
                                v0_s[:]
                                .rearrange("p (k c) -> p k c", c=65)[
                                    :, ds(tcx * 4, 4), 0:64
                                ],
                                src[:, :, 0:64],
                            )
                            nc.vector.tensor_copy(
                                v1_s[:]
                                .rearrange("p (k c) -> p k c", c=65)[
                                    :, ds(tcx * 4, 4), 0:64
                                ],
                                src[:, :, 64:128],
                            )

                    return go

                return [
                    qk_piece(wq_s, qT_s),
                    qk_piece(wk_s, kT_s),
                    v_piece(0),
                    v_piece(1),
                    v_piece(2),
                    v_piece(3),
                ]

            # ---- per-Q finalize pieces (normalize + W_O projection) ----
            # Deferred into the NEXT q-block's chunk loop as filler pieces so
            # the multi-engine chain (DVE copy -> PE bcast -> DVE recip ->
            # GpSimd mul -> DMA shift -> PE proj -> DVE copy -> DMA out)
            # overlaps a whole block instead of stalling PE/ScalarE at the
            # boundary (a >3.4us PE idle there re-throttles HAM to 1.2GHz).
            def make_finalize(Q, po0, po1):
                qsl = ts(Q, 512)
                state = {}

                def p_copy():
                    # free the PV banks: single DVE copy per head to SBUF,
                    # then broadcast the sums rows to 64 partitions (K=1
                    # matmuls) and take reciprocals -- 64 lanes instead of 1
                    nc.vector.tensor_copy(aoU0_s[:, qsl], po0[:])
                    nc.vector.tensor_copy(aoU1_s[:, qsl], po1[:])

                def p_bcast0():
                    psb0 = ps_mi.tile([64, 512], FP32, tag="mi", name="psb0")
                    nc.tensor.matmul(
                        psb0[:],
                        lhsT=ones_row[64:65, :],
                        rhs=aoU0_s[64:65, qsl],
                        start=True,
                        stop=True,
                    )
                    rbc0 = wpool.tile([64, 512], FP32, tag="bc", name="rbc0")
                    nc.vector.reciprocal_approx_fast(rbc0[:], psb0[:])
                    state["rbc0"] = rbc0

                def p_bcast1():
                    psb1 = ps_mi.tile([64, 512], FP32, tag="mi", name="psb1")
                    nc.tensor.matmul(
                        psb1[:],
                        lhsT=ones_row[64:65, :],
                        rhs=aoU1_s[64:65, qsl],
                        start=True,
                        stop=True,
                    )
                    rbc1 = wpool.tile([64, 512], FP32, tag="bc", name="rbc1")
                    nc.vector.reciprocal_approx_fast(rbc1[:], psb1[:])
                    state["rbc1"] = rbc1

                def p_norm():
                    # normalized attention-out, both heads in one [128, 512]
                    # tile (head1 lands via an SBUF->SBUF DMA partition
                    # shift) so the output projection is a single K=128
                    # matmul per 128 queries; multiplies on GpSimd (idle-ish)
                    aoT_b = wpool.tile([128, 512], BF16, tag="ao", name="aoT_b")
                    nc.gpsimd.tensor_mul(
                        aoT_b[0:64, :],
                        aoU0_s[0:64, qsl].bitcast(FP32),
                        state["rbc0"][:],
                    )
                    aoT1 = wpool.tile([64, 512], BF16, tag="ao1", name="aoT1")
                    nc.gpsimd.tensor_mul(
                        aoT1[:], aoU1_s[0:64, qsl].bitcast(FP32), state["rbc1"][:]
                    )
                    nc.sync.dma_start(aoT_b[64:128, :], aoT1[:])
                    state["aoT_b"] = aoT_b

                def p_proj(qq):
                    def go():
                        pso = ps_mi.tile([128, 512], FP32, tag="mi", name="pso")
                        nc.tensor.matmul(
                            pso[:],
                            lhsT=state["aoT_b"][:, ts(qq, 128)],
                            rhs=woT_s[:],
                            start=True,
                            stop=True,
                        )
                        osb = wpool.tile([128, 512], FP32, tag="os", name="osb")
                        nc.vector.tensor_copy(osb[:], pso[:])
                        nc.sync.dma_start(
                            outp[ds(Q * 512 + qq * 128, 128), :], osb[:]
                        )

                    return go

                return [p_copy, p_bcast0, p_bcast1, p_norm] + [
                    p_proj(qq) for qq in range(4)
                ]

            fillers = deque()
            pieces0 = make_pieces(0)
            for p in pieces0[:2]:
                p()  # q and k of chunk 0: the critical path to first scores
            # v pieces of chunk 0 are only needed by the first PV (two
            # iterations in) -- queue them as fillers ahead of chunk 1's
            fillers.extend(pieces0[2:])
            if nq > 1:
                fillers.extend(make_pieces(1))

            def pop_filler(slots_left):
                # adaptive: drain the queue evenly over the remaining slots
                n = -(-len(fillers) // max(slots_left, 1))
                for _ in range(min(n, len(fillers))):
                    fillers.popleft()()

            # ---- attention ----
            inv_sqrt_d = 1.0 / math.sqrt(64.0)
            # bit-trick exp for the DVE offload path: bf16 bit pattern of
            # exp(s*inv_sqrt_d) ~= int16(s * A7 + B7) (fp32->int16 converts
            # round-to-nearest, HW-verified; +-3% sawtooth error on ~1/4 of
            # the attention weights, and the softmax denominator uses the
            # same approximated values so normalization is consistent;
            # measured end-to-end: 3.2e-3 rel err vs 2.7e-3 all-ScalarE)
            exp_a7 = inv_sqrt_d * 128.0 / math.log(2.0)
            exp_b7 = 16256.0 - 0.0579 * 128.0
            for Q in range(nq):
                po0 = ps_pv.tile([65, 512], FP32, tag="pv0", name="po0")
                po1 = ps_pv.tile([65, 512], FP32, tag="pv1", name="po1")
                nkq = 4 * Q + 4
                pts = {}
                last_scores = None
                # software-pipelined in chunk PAIRS: both chunks' score
                # matmuls are emitted back-to-back (the row-tiled halves
                # alternate PE row groups, so each LDWEIGHTS hides behind
                # the other head's in-flight matmul), then the previous
                # pair's PV matmuls, one pair behind, so the PE never waits
                # out the ScalarE/DVE exp latency
                niter = nkq // 2 + 1
                for I in range(niter):
                    for cpar in (0, 1):
                        K = 2 * I + cpar
                        if K >= nkq:
                            continue
                        off = K * 128 - Q * 512
                        n0 = max(off, 0)
                        w = 512 - n0
                        pssc = ps_sc.tile([128, 1024], FP32, tag="sc", name="pssc")
                        nc.tensor.matmul(
                            pssc[:, n0:512],
                            lhsT=kT_s[0:64, ts(K, 128)],
                            rhs=qT_s[0:64, ds(Q * 512 + n0, w)],
                            start=True,
                            stop=True,
                        )
                        last_scores = nc.tensor.matmul(
                            pssc[:, 512 + n0 : 1024],
                            lhsT=kT_s[64:128, ts(K, 128)],
                            rhs=qT_s[64:128, ds(Q * 512 + n0, w)],
                            start=True,
                            stop=True,
                        )
                        pt = wpool.tile([128, 1024], BF16, tag="pt", name="pt", bufs=4)
                        src = pssc[:].rearrange("p (h n) -> p h n", h=2)[:, :, n0:512]
                        if off < 0 and K % 3 == 2:
                            # offload ~1/3 of the exps to the DVE (ScalarE is
                            # the critical engine); off-diagonal chunks only
                            # so the mask path stays on the ScalarE side
                            dsti = pt[:].bitcast(I16).rearrange(
                                "p (h n) -> p h n", h=2
                            )[:, :, n0:512]
                            nc.vector.tensor_scalar(
                                dsti,
                                src,
                                exp_a7,
                                exp_b7,
                                mybir.AluOpType.mult,
                                mybir.AluOpType.add,
                            )
                        else:
                            dst = pt[:].rearrange("p (h n) -> p h n", h=2)[
                                :, :, n0:512
                            ]
                            nc.scalar.activation(dst, src, AF.Exp, scale=inv_sqrt_d)
                        if off >= 0:
                            # zero the not-yet-valid triangle on the (idle)
                            # GpSimd engine, off the scores->exp chain
                            nc.gpsimd.tensor_mul(
                                pt[:, ds(n0, 128)], pt[:, ds(n0, 128)], mask_s[:]
                            )
                            nc.gpsimd.tensor_mul(
                                pt[:, ds(512 + n0, 128)],
                                pt[:, ds(512 + n0, 128)],
                                mask_s[:],
                            )
                        pts[K] = (pt, n0, w)
                    # filler pieces (QKV projection for block Q+1, finalize
                    # work for block Q-1) spread over the block so every
                    # cross-engine dependency has a whole block of slack and
                    # the PE queue never blocks on a remote chain
                    pop_filler(2 * (niter - I))
                    for cpar in (0, 1):
                        Kp = 2 * (I - 1) + cpar
                        if not (0 <= Kp < nkq):
                            continue
                        pt_p, n0_p, w_p = pts.pop(Kp)
                        st = Kp == 0
                        sp = Kp == nkq - 1
                        pv0_mm = nc.tensor.matmul(
                            po0[0:65, ds(n0_p, w_p)],
                            lhsT=v0_s[:, ds(Kp * 65, 65)],
                            rhs=pt_p[:, ds(n0_p, w_p)],
                            start=st,
                            stop=sp,
                            skip_group_check=True,
                        )
                        if cpar == 0 and I < niter - 1 and last_scores is not None:
                            # order-only edge: keep the PV pairs AFTER the
                            # next chunks' scores on the PE queue so the exp
                            # latency is hidden behind PE work
                            add_dep_helper(
                                pv0_mm.ins,
                                last_scores.ins,
                                sync=False,
                                reason="pipeline skew",
                            )
                        nc.tensor.matmul(
                            po1[0:65, ds(n0_p, w_p)],
                            lhsT=v1_s[:, ds(Kp * 65, 65)],
                            rhs=pt_p[:, ds(512 + n0_p, w_p)],
                            start=st,
                            stop=sp,
                            skip_group_check=True,
                        )
                    pop_filler(2 * (niter - I) - 1)
                # queue this block's finalize + block Q+2's projection as
                # fillers for block Q+1; the last block finalizes inline
                if Q + 1 < nq:
                    fillers.extend(make_finalize(Q, po0, po1))
                    if Q + 2 < nq:
                        fillers.extend(make_pieces(Q + 2))
                else:
                    fillers.extend(make_finalize(Q, po0, po1))
                    while fillers:
                        fillers.popleft()()
    nc.compile()
    return nc


def make_in_maps(x, W_QKV, W_O, t=T, n_cores=8):
    x = np.ascontiguousarray(np.asarray(x, dtype=np.float32))
    W_QKV = np.asarray(W_QKV, dtype=np.float32)
    W_O = np.asarray(W_O, dtype=np.float32)
    B = x.shape[0]
    bf16 = ml_dtypes.bfloat16

    def prearrange(w):
        # w: [128 hd, 512 dm] slice of a linear weight; SBUF wants
        # [p, d*128 + c] = w.T[d*128 + p, c] so the DMA is contiguous
        return np.ascontiguousarray(
            w.T.reshape(4, 128, 128).transpose(1, 0, 2).reshape(128, 512)
        ).astype(bf16)

    xTs = [np.ascontiguousarray(x[b, :t].T).astype(bf16) for b in range(B)]
    in_maps = []
    for c in range(n_cores):
        b = c // 4
        g = c % 4
        hs = slice(2 * g * 64, 2 * g * 64 + 128)
        in_maps.append(
            {
                "xT": xTs[b],
                "wq": prearrange(W_QKV[0:512][hs]),
                "wk": prearrange(W_QKV[512:1024][hs]),
                "wv": prearrange(W_QKV[1024:1536][hs]),
                "woT": np.ascontiguousarray(W_O[:, hs].T).astype(bf16),
            }
        )
    return in_maps


def kernel(x, W_QKV, W_O):
    global LAST_RESULTS
    x = np.asarray(x, dtype=np.float32)
    B, t, _ = x.shape
    nc = build_program(t)
    in_maps = make_in_maps(x, W_QKV, W_O, t=t)
    res = run_bass_kernel_spmd(
        nc, in_maps, core_ids=list(range(8)), trace=TRACE
    )
    LAST_RESULTS = res
    parts = [r["outp"] for r in res.results]
    out = np.empty((B, t, DM), dtype=np.float32)
    for b in range(B):
        acc = np.zeros((t, DM), dtype=np.float64)
        for g in range(4):
            acc += parts[b * 4 + g]
        out[b] = acc.astype(np.float32)
    return out
